# revision 3
# baseline (speedup 1.0000x reference)
"""Distributed Trainium2 kernel for nn_Attention_68719477187.

RoPE + causal GQA attention (B=2, S=2048, DIM=2048, 32 q heads / 8 kv heads,
head_dim 64) on 8 NeuronCores: DP=2 over batch x TP=4 over head groups.

Per core (b = core//4, G = core%4): 8 q heads / 2 kv heads of batch b.
  1. qkv.T = w{q,k,v}T.T @ x_b.T (contraction over model dim on partitions)
  2. RoPE applied in transposed layout; head_dim pre-permuted (evens, odds)
     on the host so rotation pairs become contiguous 32-partition blocks.
  3. scores.T tiles (k on partitions, q on free) -> exp (no max subtraction;
     scores are O(5) so fp32 exp is safe) -> causal mask by 0/1 multiply ->
     AV matmul with a ones-column appended to V so the softmax denominator
     falls out of the same matmul.
  4. AllGather attention outputs (bf16) within each batch group of 4 cores,
     then each core computes a 512-column slice of the final wo projection.

Compute in bf16 (fp32 PSUM accumulation), output fp32.
"""

import sys

if "/opt/trn_rl_repo" not in sys.path:
    sys.path.insert(0, "/opt/trn_rl_repo")

import numpy as np
import ml_dtypes

from concourse import bacc, tile, mybir
from concourse.bass_utils import run_bass_kernel_spmd

BF16 = ml_dtypes.bfloat16

S = 2048          # sequence length
D = 2048          # model dim
HD = 64           # head dim
NQL = 8           # local q heads
NKVL = 2          # local kv heads
QC = 512          # q chunk (matmul free dim)
NSC = S // QC     # 4 seq chunks
NKD = D // 128    # 16 contraction tiles
NKT = S // 128    # 16 key tiles
SCALE = HD ** -0.5

_NC = None


def _build():
    nc = bacc.Bacc("TRN2", target_bir_lowering=False, debug=False, num_devices=8)
    BF = mybir.dt.bfloat16
    F32 = mybir.dt.float32
    EXP = mybir.ActivationFunctionType.Exp

    xT = nc.declare_dram_parameter("xT", [D, S], BF, isOutput=False)
    wqT = nc.declare_dram_parameter("wqT", [D, NQL * HD], BF, isOutput=False)
    wkT = nc.declare_dram_parameter("wkT", [D, NKVL * HD], BF, isOutput=False)
    wvT = nc.declare_dram_parameter("wvT", [D, NKVL * HD], BF, isOutput=False)
    woT = nc.declare_dram_parameter("woT", [D, 512], BF, isOutput=False)
    cosS = nc.declare_dram_parameter("cosS", [128, S], BF, isOutput=False)
    sinS = nc.declare_dram_parameter("sinS", [128, S], BF, isOutput=False)
    idn = nc.declare_dram_parameter("idn", [128, 128], BF, isOutput=False)
    mask = nc.declare_dram_parameter("mask", [128, 4, QC], BF, isOutput=False)
    out = nc.declare_dram_parameter("out", [512, S], F32, isOutput=True)

    with tile.TileContext(nc) as tc:
        with (
            tc.tile_pool(name="wpool", bufs=1) as wpool,
            tc.tile_pool(name="pers", bufs=1) as pers,
            tc.tile_pool(name="dram", bufs=1, space="DRAM") as dram,
        ):
            # ---- persistent weights / constants ----
            wq_sb = wpool.tile([128, NKD, 512], BF, name="wq_sb", tag="wq_sb")
            wk_sb = wpool.tile([128, NKD, 128], BF, name="wk_sb", tag="wk_sb")
            wv_sb = wpool.tile([128, NKD, 128], BF, name="wv_sb", tag="wv_sb")
            wo_sb = wpool.tile([128, NKD, 512], BF, name="wo_sb", tag="wo_sb")
            cos_sb = wpool.tile([128, S], BF, name="cos_sb", tag="cos_sb")
            sin_sb = wpool.tile([128, S], BF, name="sin_sb", tag="sin_sb")
            id_sb = wpool.tile([128, 128], BF, name="id_sb", tag="id_sb")
            mask_sb = wpool.tile([128, 4, QC], BF, name="mask_sb", tag="mask_sb")
            for kd in range(NKD):
                nc.sync.dma_start(wq_sb[:, kd, :], wqT[kd * 128:(kd + 1) * 128, :])
                nc.sync.dma_start(wk_sb[:, kd, :], wkT[kd * 128:(kd + 1) * 128, :])
                nc.sync.dma_start(wv_sb[:, kd, :], wvT[kd * 128:(kd + 1) * 128, :])
                nc.sync.dma_start(wo_sb[:, kd, :], woT[kd * 128:(kd + 1) * 128, :])
            nc.sync.dma_start(cos_sb[:], cosS[:])
            nc.sync.dma_start(sin_sb[:], sinS[:])
            nc.sync.dma_start(id_sb[:], idn[:])
            nc.sync.dma_start(mask_sb[:], mask[:])

            # ---- persistent activations ----
            # qT[rt][sc]: (128, 512) = q heads (2rt, 2rt+1), [e,o|e,o] layout
            qT = [[pers.tile([128, QC], BF, name=f"qT_{rt}_{sc}", tag=f"qT_{rt}_{sc}")
                   for sc in range(NSC)] for rt in range(4)]
            # kdup[j][sc]: kv head j duplicated on both partition halves
            kdup = [[pers.tile([128, QC], BF, name=f"kd_{j}_{sc}", tag=f"kd_{j}_{sc}")
                     for sc in range(NSC)] for j in range(NKVL)]
            # vaug[kt]: (128, 2, 65) = per kv head: V block + ones column
            vaug = [pers.tile([128, 2, 65], BF, name=f"va_{kt}", tag=f"va_{kt}")
                    for kt in range(NKT)]
            attnT = [pers.tile([128, S], BF, name=f"at_{rt}", tag=f"at_{rt}")
                     for rt in range(4)]
            ag_in = dram.tile([NQL * HD, S], BF, name="ag_in")
            ag_out = dram.tile([4 * NQL * HD, S], BF, name="ag_out")

            # ================= phase 1: qkv projection + rope =================
            with (
                tc.tile_pool(name="xpool", bufs=24) as xpool,
                tc.tile_pool(name="qkvps", bufs=4, space="PSUM") as qkvps,
                tc.tile_pool(name="trps", bufs=2, space="PSUM") as trps,
                tc.tile_pool(name="rtmp", bufs=3) as rtmp,
            ):
                for sc in range(NSC):
                    xt = []
                    for kd in range(NKD):
                        t = xpool.tile([128, QC], BF, name="xt", tag="xt")
                        nc.sync.dma_start(t[:], xT[kd * 128:(kd + 1) * 128,
                                                   sc * QC:(sc + 1) * QC])
                        xt.append(t)

                    cslice = cos_sb[:, sc * QC:(sc + 1) * QC]
                    sslice = sin_sb[:, sc * QC:(sc + 1) * QC]

                    for rt in range(5):  # 0..3: q row tiles; 4: k row tile
                        ps = qkvps.tile([128, QC], F32, name="qkv_ps", tag="qkv_ps")
                        for kd in range(NKD):
                            lhsT = (wq_sb[:, kd, rt * 128:(rt + 1) * 128]
                                    if rt < 4 else wk_sb[:, kd, :])
                            nc.tensor.matmul(ps[:], lhsT, xt[kd][:],
                                             start=(kd == 0), stop=(kd == NKD - 1))
                        raw = rtmp.tile([128, QC], BF, name="raw", tag="raw")
                        nc.scalar.copy(raw[:], ps[:])
                        # rope: out = raw*cos + swap32(raw)*sin_signed
                        rot = rtmp.tile([128, QC], BF, name="rot", tag="rot")
                        for b32 in range(4):
                            src = (b32 ^ 1) * 32
                            nc.vector.tensor_copy(rot[b32 * 32:(b32 + 1) * 32, :],
                                                  raw[src:src + 32, :])
                        t1 = rtmp.tile([128, QC], BF, name="t1", tag="t1")
                        nc.vector.tensor_mul(t1[:], raw[:], cslice)
                        nc.vector.tensor_mul(rot[:], rot[:], sslice)
                        if rt < 4:
                            nc.vector.tensor_add(qT[rt][sc][:], t1[:], rot[:])
                        else:
                            kr = rtmp.tile([128, QC], BF, name="kr", tag="kr")
                            nc.vector.tensor_add(kr[:], t1[:], rot[:])
                            for j in range(NKVL):
                                src = kr[j * 64:(j + 1) * 64, :]
                                nc.vector.tensor_copy(kdup[j][sc][0:64, :], src)
                                nc.vector.tensor_copy(kdup[j][sc][64:128, :], src)

                    # V row tile: no rope; transpose to (seq, dim) + ones col
                    vps = qkvps.tile([128, QC], F32, name="qkv_ps", tag="qkv_ps")
                    for kd in range(NKD):
                        nc.tensor.matmul(vps[:], wv_sb[:, kd, :], xt[kd][:],
                                         start=(kd == 0), stop=(kd == NKD - 1))
                    vraw = rtmp.tile([128, QC], BF, name="vraw", tag="vraw")
                    nc.scalar.copy(vraw[:], vps[:])
                    for tt in range(4):
                        kt = sc * 4 + tt
                        tp = trps.tile([128, 128], BF, name="tp", tag="tp")
                        nc.tensor.transpose(tp[:], vraw[:, tt * 128:(tt + 1) * 128],
                                            id_sb[:])
                        for j in range(NKVL):
                            nc.vector.tensor_copy(vaug[kt][:, j, 0:64],
                                                  tp[:, j * 64:(j + 1) * 64])
                            nc.gpsimd.memset(vaug[kt][:, j, 64:65], 1.0)

            # ================= phase 2: attention =================
            with (
                tc.tile_pool(name="stps", bufs=3, space="PSUM") as stps,
                tc.tile_pool(name="avps", bufs=2, space="PSUM") as avps,
                tc.tile_pool(name="ppool", bufs=4) as ppool,
                tc.tile_pool(name="npool", bufs=2) as npool,
            ):
                for qc in range(NSC):
                    nkt = 4 * (qc + 1)  # causal: only key tiles up to chunk end
                    for h in range(NQL):
                        rt, half, j = h // 2, h % 2, h // 4
                        qsl = qT[rt][qc][half * 64:(half + 1) * 64, :]
                        av = avps.tile([65, QC], F32, name="av", tag="av")
                        for kt in range(nkt):
                            ksl = kdup[j][kt // 4][half * 64:(half + 1) * 64,
                                                   (kt % 4) * 128:(kt % 4 + 1) * 128]
                            st = stps.tile([128, QC], F32, name="st", tag="st")
                            nc.tensor.matmul(st[:], ksl, qsl, start=True, stop=True)
                            p = ppool.tile([128, QC], BF, name="p", tag="p")
                            nc.scalar.activation(p[:], st[:], EXP, scale=SCALE)
                            m = kt - 4 * qc
                            if m >= 0:  # diagonal tile -> causal 0/1 mask
                                nc.vector.tensor_mul(p[:], p[:], mask_sb[:, m, :])
                            nc.tensor.matmul(av[:], vaug[kt][:, j, :], p[:],
                                             start=(kt == 0), stop=(kt == nkt - 1))
                        recip = npool.tile([1, QC], F32, name="recip", tag="recip")
                        nc.vector.reciprocal(recip[:], av[64:65, :])
                        rb = npool.tile([64, QC], F32, name="rb", tag="rb")
                        nc.gpsimd.partition_broadcast(rb[:], recip[:])
                        nc.vector.tensor_mul(
                            attnT[rt][half * 64:(half + 1) * 64, qc * QC:(qc + 1) * QC],
                            av[0:64, :], rb[:])

            for rt in range(4):
                nc.sync.dma_start(ag_in[rt * 128:(rt + 1) * 128, :], attnT[rt][:])
            nc.gpsimd.collective_compute(
                "AllGather", mybir.AluOpType.bypass,
                replica_groups=[[0, 1, 2, 3], [4, 5, 6, 7]],
                ins=[ag_in.opt()], outs=[ag_out.opt()])

            # ================= phase 3: output projection =================
            with (
                tc.tile_pool(name="agpool", bufs=20) as agp,
                tc.tile_pool(name="wops", bufs=4, space="PSUM") as wops,
                tc.tile_pool(name="opool", bufs=3) as opool,
            ):
                for qn in range(NSC):
                    agt = []
                    for kd in range(NKD):
                        t = agp.tile([128, QC], BF, name="agt", tag="agt")
                        nc.sync.dma_start(t[:], ag_out[kd * 128:(kd + 1) * 128,
                                                       qn * QC:(qn + 1) * QC])
                        agt.append(t)
                    for oc in range(4):
                        ps = wops.tile([128, QC], F32, name="wo_ps", tag="wo_ps")
                        for kd in range(NKD):
                            nc.tensor.matmul(ps[:], wo_sb[:, kd, oc * 128:(oc + 1) * 128],
                                             agt[kd][:],
                                             start=(kd == 0), stop=(kd == NKD - 1))
                        ot = opool.tile([128, QC], F32, name="ot", tag="ot")
                        nc.scalar.copy(ot[:], ps[:])
                        nc.sync.dma_start(out[oc * 128:(oc + 1) * 128,
                                              qn * QC:(qn + 1) * QC], ot[:])

    nc.compile()
    return nc


def _get_nc():
    global _NC
    if _NC is None:
        _NC = _build()
    return _NC


def _prepare_in_maps(x, freqs_cis, wqkv, wo):
    x = np.asarray(x)
    freqs_cis = np.asarray(freqs_cis)
    wqkv = np.asarray(wqkv)
    wo = np.asarray(wo)

    perm = np.concatenate([np.arange(0, HD, 2), np.arange(1, HD, 2)])
    cos = np.ascontiguousarray(freqs_cis[:, :, 0].T)  # (32, S)
    sin = np.ascontiguousarray(freqs_cis[:, :, 1].T)
    cosS = np.concatenate([cos, cos, cos, cos], axis=0).astype(BF16)
    sinS = np.concatenate([-sin, sin, -sin, sin], axis=0).astype(BF16)
    idn = np.eye(128, dtype=BF16)
    p_i = np.arange(128)[:, None]
    f_i = np.arange(QC)[None, :]
    mask = np.stack([(f_i >= p_i + 128 * m) for m in range(4)], axis=1).astype(BF16)

    xTs = [np.ascontiguousarray(x[b].T).astype(BF16) for b in range(2)]

    in_maps = []
    for c in range(8):
        b, G = c // 4, c % 4
        qrows = np.concatenate([(8 * G + h) * HD + perm for h in range(NQL)])
        krows = np.concatenate([D + (2 * G + j) * HD + perm for j in range(NKVL)])
        vrows = np.concatenate([D + 512 + (2 * G + j) * HD + np.arange(HD)
                                for j in range(NKVL)])
        in_maps.append({
            "xT": xTs[b],
            "wqT": np.ascontiguousarray(wqkv[qrows, :].T).astype(BF16),
            "wkT": np.ascontiguousarray(wqkv[krows, :].T).astype(BF16),
            "wvT": np.ascontiguousarray(wqkv[vrows, :].T).astype(BF16),
            "woT": np.ascontiguousarray(wo[512 * G:512 * (G + 1), :].T).astype(BF16),
            "cosS": cosS,
            "sinS": sinS,
            "idn": idn,
            "mask": mask,
        })
    return in_maps


def kernel(x, freqs_cis, wqkv, wo, _trace=False):
    in_maps = _prepare_in_maps(x, freqs_cis, wqkv, wo)
    res = run_bass_kernel_spmd(_get_nc(), in_maps, core_ids=list(range(8)),
                               trace=_trace)

    outf = np.empty((2, S, D), np.float32)
    for c in range(8):
        b, G = c // 4, c % 4
        outf[b, :, 512 * G:512 * (G + 1)] = res.results[c]["out"].T
    if _trace:
        kernel.last_exec_time_ns = res.exec_time_ns
        kernel.last_results = res
    return outf


# revision 22
# speedup vs baseline: 159.4684x; 159.4684x over previous
"""Distributed Trainium2 kernel for nn_Attention_68719477187.

RoPE + causal GQA attention (B=2, S=2048, DIM=2048, 32 q heads / 8 kv heads,
head_dim 64) on 8 NeuronCores: DP=2 over batch x TP=4 over head groups.

Per core (b = core//4, G = core%4): 8 q heads / 2 kv heads of batch b.
  1. qkv.T = w{q,k,v}T.T @ x_b.T (contraction over model dim on partitions)
  2. RoPE applied in transposed layout; head_dim pre-permuted (evens, odds)
     on the host so rotation pairs become contiguous 32-partition blocks.
  3. scores.T tiles (k on partitions, q on free) -> exp (no max subtraction;
     scores are O(5) so fp32 exp is safe) -> causal mask by 0/1 multiply ->
     AV matmul with a ones-column appended to V so the softmax denominator
     falls out of the same matmul.
  4. AllGather attention outputs (bf16, chunked per 512 seq positions and
     pipelined behind later attention chunks) within each batch group of 4
     cores, then each core computes a 512-column slice of wo.

Phases are interleaved per sequence chunk sc: qkv(sc) -> attention(qc=sc)
-> AllGather(sc) -> wo(sc-1), so PE matmul work overlaps the ACT-bound
softmax and the collectives.

Compute in bf16 (fp32 PSUM accumulation), output fp32.
"""

import sys

if "/opt/trn_rl_repo" not in sys.path:
    sys.path.insert(0, "/opt/trn_rl_repo")

import numpy as np
import ml_dtypes

from concourse import bacc, tile, mybir
from concourse.bass_utils import run_bass_kernel_spmd

BF16 = ml_dtypes.bfloat16

S = 2048          # sequence length
D = 2048          # model dim
HD = 64           # head dim
NQL = 8           # local q heads
NKVL = 2          # local kv heads
QC = 512          # q chunk (matmul free dim)
NSC = S // QC     # 4 seq chunks
NKD = D // 128    # 16 contraction tiles
NKT = S // 128    # 16 key tiles
SCALE = HD ** -0.5

_NC = None


def _build(_no_cc=False):
    import os
    _bufs = os.environ.get("KBUFS", "")  # "mm,st,av,pp" override for tuning
    mm_b, st_b, av_b, pp_b = ([int(v) for v in _bufs.split(",")]
                              if _bufs else [1, 4, 2, 4])
    nc = bacc.Bacc("TRN2", target_bir_lowering=False, debug=False, num_devices=8)
    BF = mybir.dt.bfloat16
    F32 = mybir.dt.float32
    EXP = mybir.ActivationFunctionType.Exp

    xT = nc.declare_dram_parameter("xT", [D, S], BF, isOutput=False)
    wqT = nc.declare_dram_parameter("wqT", [D, NQL * HD], BF, isOutput=False)
    wkT = nc.declare_dram_parameter("wkT", [D, NKVL * HD], BF, isOutput=False)
    wvT = nc.declare_dram_parameter("wvT", [D, NKVL * HD], BF, isOutput=False)
    woT = nc.declare_dram_parameter("woT", [D, 512], BF, isOutput=False)
    cosS = nc.declare_dram_parameter("cosS", [128, S], F32, isOutput=False)
    sinS = nc.declare_dram_parameter("sinS", [128, S], F32, isOutput=False)
    mask = nc.declare_dram_parameter("mask", [128, 4, QC], BF, isOutput=False)
    out = nc.declare_dram_parameter("out", [512, S], F32, isOutput=True)

    with tile.TileContext(nc) as tc:
        with (
            tc.tile_pool(name="wpool", bufs=1) as wpool,
            tc.tile_pool(name="pers", bufs=1) as pers,
            tc.tile_pool(name="dram", bufs=1, space="DRAM") as dram,
            tc.tile_pool(name="xpool", bufs=3) as xpool,
            tc.tile_pool(name="rtmp", bufs=2) as rtmp,
            tc.tile_pool(name="ppool", bufs=pp_b) as ppool,
            tc.tile_pool(name="npool", bufs=2) as npool,
            tc.tile_pool(name="apool", bufs=2) as apool,
            tc.tile_pool(name="agp", bufs=1) as agp,
            tc.tile_pool(name="opool", bufs=2) as opool,
            tc.tile_pool(name="mmps", bufs=mm_b, space="PSUM") as mmps,
            tc.tile_pool(name="stps", bufs=st_b, space="PSUM") as stps,
            tc.tile_pool(name="avps", bufs=av_b, space="PSUM") as avps,
            tc.tile_pool(name="wops", bufs=1, space="PSUM") as wops,
        ):
            # ---- persistent weights / constants (one 3D DMA each) ----
            wq_sb = [wpool.tile([128, NKD // 2, 512], BF, name=f"wq_sb{h}",
                                tag=f"wq_sb{h}") for h in range(2)]
            wk_sb = wpool.tile([128, NKD, 128], BF, name="wk_sb", tag="wk_sb")
            wv_sb = wpool.tile([128, NKD, 128], BF, name="wv_sb", tag="wv_sb")
            wo_sb = wpool.tile([128, NKD, 512], BF, name="wo_sb", tag="wo_sb")
            cos_sb = wpool.tile([128, S], F32, name="cos_sb", tag="cos_sb")
            sin_sb = wpool.tile([128, S], F32, name="sin_sb", tag="sin_sb")
            mask_sb = wpool.tile([128, 4, QC], BF, name="mask_sb", tag="mask_sb")
            wqTr = wqT.rearrange("(k p) c -> p k c", p=128)

            # ---- persistent activations ----
            qT = [[pers.tile([128, QC], BF, name=f"qT_{rt}_{sc}", tag=f"qT_{rt}_{sc}")
                   for sc in range(NSC)] for rt in range(4)]
            kdup = [[pers.tile([128, QC], BF, name=f"kd_{j}_{sc}", tag=f"kd_{j}_{sc}")
                     for sc in range(NSC)] for j in range(NKVL)]
            vaug = [pers.tile([128, 2, 65], BF, name=f"va_{kt}", tag=f"va_{kt}")
                    for kt in range(NKT)]
            ag_in = [dram.tile([NQL * HD, QC], BF, name=f"ag_in_{qc}")
                     for qc in range(NSC)]
            ag_out = [dram.tile([4 * NQL * HD, QC], BF, name=f"ag_out_{qc}")
                      for qc in range(NSC)]

            xTr = xT.rearrange("(k p) s -> p k s", p=128)
            # hoist x loads so later SP-queue DMAs (which wait on the
            # collectives) never block them; the last chunk is prefetched
            # during qkv(2), still ahead of any AG-dependent DMA in SP order.
            # wq / x chunk 0 are loaded in halves so the first matmuls start
            # after ~1MB of DMA instead of 4MB.
            xts = {}

            def load_x(sc):
                halves = []
                for h in range(2):
                    xt = xpool.tile([128, NKD // 2, QC], BF, name="xt", tag="xt")
                    nc.sync.dma_start(
                        xt[:], xTr[:, h * 8:(h + 1) * 8, sc * QC:(sc + 1) * QC])
                    halves.append(xt)
                xts[sc] = halves

            nc.sync.dma_start(wq_sb[0][:], wqTr[:, 0:8, :])
            load_x(0)
            nc.sync.dma_start(wq_sb[1][:], wqTr[:, 8:16, :])
            nc.sync.dma_start(wk_sb[:], wkT.rearrange("(k p) c -> p k c", p=128))
            nc.sync.dma_start(wv_sb[:], wvT.rearrange("(k p) c -> p k c", p=128))
            nc.sync.dma_start(cos_sb[:], cosS[:])
            nc.sync.dma_start(sin_sb[:], sinS[:])
            for sc in range(1, NSC - 1):
                load_x(sc)
            nc.sync.dma_start(mask_sb[:], mask[:])
            nc.sync.dma_start(wo_sb[:], woT.rearrange("(k p) c -> p k c", p=128))

            def qkv_phase(sc):
                if sc == 2:
                    load_x(3)
                xt = xts[sc]
                cslice = cos_sb[:, sc * QC:(sc + 1) * QC]
                sslice = sin_sb[:, sc * QC:(sc + 1) * QC]
                for rt in range(5):  # 0..3: q row tiles; 4: k row tile
                    ps = mmps.tile([128, QC], F32, name="mm_ps", tag="mm_ps")
                    for kd in range(NKD):
                        lhsT = (wq_sb[kd // 8][:, kd % 8, rt * 128:(rt + 1) * 128]
                                if rt < 4 else wk_sb[:, kd, :])
                        nc.tensor.matmul(ps[:], lhsT, xt[kd // 8][:, kd % 8, :],
                                         start=(kd == 0), stop=(kd == NKD - 1))
                    # rope in fp32 (bf16 only at the final q/k write):
                    # out = raw*cos + swap32(raw)*sin_signed
                    raw = rtmp.tile([128, QC], F32, name="raw", tag="raw")
                    nc.scalar.copy(raw[:], ps[:])
                    rot = rtmp.tile([128, QC], F32, name="rot", tag="rot")
                    for b32 in range(4):
                        src = (b32 ^ 1) * 32
                        nc.gpsimd.tensor_copy(rot[b32 * 32:(b32 + 1) * 32, :],
                                              raw[src:src + 32, :])
                    t1 = rtmp.tile([128, QC], F32, name="t1", tag="t1")
                    nc.vector.tensor_mul(t1[:], raw[:], cslice)
                    nc.vector.tensor_mul(rot[:], rot[:], sslice)
                    if rt < 4:
                        nc.vector.tensor_add(qT[rt][sc][:], t1[:], rot[:])
                    else:
                        kr = rtmp.tile([128, QC], BF, name="kr", tag="kr")
                        nc.vector.tensor_add(kr[:], t1[:], rot[:])
                        for j in range(NKVL):
                            src = kr[j * 64:(j + 1) * 64, :]
                            nc.vector.tensor_copy(kdup[j][sc][0:64, :], src)
                            nc.vector.tensor_copy(kdup[j][sc][64:128, :], src)
                # V computed directly in natural (seq, dim) orientation:
                # lhsT = x.T seq-slice, rhs = wv.T -> out (seq, 2*64) + ones col
                for tt in range(4):
                    kt = sc * 4 + tt
                    vp = stps.tile([128, 128], F32, name="st", tag="st")
                    for kd in range(NKD):
                        nc.tensor.matmul(vp[:],
                                         xt[kd // 8][:, kd % 8, tt * 128:(tt + 1) * 128],
                                         wv_sb[:, kd, :],
                                         start=(kd == 0), stop=(kd == NKD - 1))
                    for j in range(NKVL):
                        nc.vector.tensor_copy(vaug[kt][:, j, 0:64],
                                              vp[:, j * 64:(j + 1) * 64])
                        nc.gpsimd.memset(vaug[kt][:, j, 64:65], 1.0)

            def attn_phase(qc):
                # attention outputs staged in one tile: (128, rt, seq-chunk)
                atile = apool.tile([128, 4, QC], BF, name="atile", tag="atile")
                nkt = 4 * (qc + 1)  # causal: only key tiles up to chunk end
                for rt in range(4):  # head pair (2rt, 2rt+1); shared kv head
                    j = rt // 2
                    avs = [avps.tile([65, QC], F32, name="av", tag="av")
                           for _ in range(2)]
                    for kt in range(nkt):
                        kb = (kt % 4) * 128
                        # diagonal k-tiles only need q columns >= 128*m
                        # (everything left of that is strictly above the
                        # causal diagonal); qo is the q-column offset
                        m = kt - 4 * qc
                        qo = 128 * m if m > 0 else 0
                        n = QC - qo
                        ps_pair = []
                        for half in range(2):
                            # operands at partition base 64*half -> the two
                            # K=64 matmuls run in different PE row groups
                            lo, hi = half * 64, half * 64 + 64
                            st = stps.tile([128, QC], F32, name="st", tag="st")
                            nc.tensor.matmul(st[:, 0:n],
                                             kdup[j][kt // 4][lo:hi, kb:kb + 128],
                                             qT[rt][qc][lo:hi, qo:QC],
                                             start=True, stop=True)
                            p = ppool.tile([128, QC], BF, name="p", tag="p")
                            nc.scalar.activation(p[:, 0:n], st[:, 0:n], EXP,
                                                 scale=SCALE)
                            if m >= 0:  # diagonal tile -> triangular 0/1 mask
                                nc.vector.tensor_mul(p[:, 0:n], p[:, 0:n],
                                                     mask_sb[:, 0, 0:n])
                            ps_pair.append(p)
                        for half in range(2):
                            nc.tensor.matmul(avs[half][:, qo:QC],
                                             vaug[kt][:, j, :],
                                             ps_pair[half][:, 0:n],
                                             start=(kt == 0), stop=(kt == nkt - 1))
                    for half in range(2):
                        av = avs[half]
                        recip = npool.tile([1, QC], F32, name="recip", tag="recip")
                        nc.vector.reciprocal(recip[:], av[64:65, :])
                        rb = npool.tile([64, QC], F32, name="rb", tag="rb")
                        nc.gpsimd.partition_broadcast(rb[:], recip[:])
                        nc.vector.tensor_mul(
                            atile[half * 64:(half + 1) * 64, rt, :],
                            av[0:64, :], rb[:])
                nc.gpsimd.dma_start(
                    ag_in[qc].rearrange("(r p) s -> p r s", p=128), atile[:])
                if _no_cc:
                    # sim-only mode: local copy instead of the collective, to
                    # measure compute-schedule quality without the cost
                    # model's (pessimistic) collective pricing
                    for r in range(4):
                        nc.gpsimd.dma_start(
                            ag_out[qc][r * 512:(r + 1) * 512, :], ag_in[qc][:])
                else:
                    nc.gpsimd.collective_compute(
                        "AllGather", mybir.AluOpType.bypass,
                        replica_groups=[[0, 1, 2, 3], [4, 5, 6, 7]],
                        ins=[ag_in[qc].opt()], outs=[ag_out[qc].opt()])

            def wo_phase(qn):
                agt = agp.tile([128, NKD, QC], BF, name="agt", tag="agt")
                nc.sync.dma_start(agt[:],
                                  ag_out[qn].rearrange("(k p) s -> p k s", p=128))
                for oc in range(4):
                    ps = wops.tile([128, QC], F32, name="wo_ps", tag="wo_ps")
                    for kd in range(NKD):
                        nc.tensor.matmul(ps[:], wo_sb[:, kd, oc * 128:(oc + 1) * 128],
                                         agt[:, kd, :],
                                         start=(kd == 0), stop=(kd == NKD - 1))
                    ot = opool.tile([128, QC], F32, name="ot", tag="ot")
                    nc.scalar.copy(ot[:], ps[:])
                    nc.sync.dma_start(out[oc * 128:(oc + 1) * 128,
                                            qn * QC:(qn + 1) * QC], ot[:])

            for sc in range(NSC):
                qkv_phase(sc)
                attn_phase(sc)
                if sc >= 2:
                    wo_phase(sc - 2)
            wo_phase(NSC - 2)
            wo_phase(NSC - 1)

    nc.compile()
    return nc


def _get_nc():
    global _NC
    if _NC is None:
        _NC = _build()
    return _NC


def _prepare_in_maps(x, freqs_cis, wqkv, wo):
    x = np.asarray(x)
    freqs_cis = np.asarray(freqs_cis)
    wqkv = np.asarray(wqkv)
    wo = np.asarray(wo)

    perm = np.concatenate([np.arange(0, HD, 2), np.arange(1, HD, 2)])
    cos = np.ascontiguousarray(freqs_cis[:, :, 0].T)  # (32, S)
    sin = np.ascontiguousarray(freqs_cis[:, :, 1].T)
    cosS = np.ascontiguousarray(np.concatenate([cos, cos, cos, cos], axis=0),
                                dtype=np.float32)
    sinS = np.ascontiguousarray(np.concatenate([-sin, sin, -sin, sin], axis=0),
                                dtype=np.float32)
    p_i = np.arange(128)[:, None]
    f_i = np.arange(QC)[None, :]
    mask = np.stack([(f_i >= p_i + 128 * m) for m in range(4)], axis=1).astype(BF16)

    xTs = [np.ascontiguousarray(x[b].T).astype(BF16) for b in range(2)]

    in_maps = []
    for c in range(8):
        b, G = c // 4, c % 4
        qrows = np.concatenate([(8 * G + h) * HD + perm for h in range(NQL)])
        krows = np.concatenate([D + (2 * G + j) * HD + perm for j in range(NKVL)])
        vrows = np.concatenate([D + 512 + (2 * G + j) * HD + np.arange(HD)
                                for j in range(NKVL)])
        in_maps.append({
            "xT": xTs[b],
            "wqT": np.ascontiguousarray(wqkv[qrows, :].T).astype(BF16),
            "wkT": np.ascontiguousarray(wqkv[krows, :].T).astype(BF16),
            "wvT": np.ascontiguousarray(wqkv[vrows, :].T).astype(BF16),
            "woT": np.ascontiguousarray(wo[512 * G:512 * (G + 1), :].T).astype(BF16),
            "cosS": cosS,
            "sinS": sinS,
            "mask": mask,
        })
    return in_maps


def kernel(x, freqs_cis, wqkv, wo, _trace=False):
    in_maps = _prepare_in_maps(x, freqs_cis, wqkv, wo)
    res = run_bass_kernel_spmd(_get_nc(), in_maps, core_ids=list(range(8)),
                               trace=_trace)

    outf = np.empty((2, S, D), np.float32)
    for c in range(8):
        b, G = c // 4, c % 4
        outf[b, :, 512 * G:512 * (G + 1)] = res.results[c]["out"].T
    if _trace:
        kernel.last_exec_time_ns = res.exec_time_ns
        kernel.last_results = res
    return outf
